# revision 72
# baseline (speedup 1.0000x reference)
"""GNN message-passing kernel for Trainium2 (8 NeuronCores, SPMD).

Strategy (edge-parallel by destination):
  * Host sorts edges by destination node, assigns 128-node blocks to
    (core, window-slot) pairs so per-slot edge counts are balanced across
    cores (one shared compile-time tile schedule for all 8 cores).
  * Host pre-gathers x[row] / edge_attr per edge UNSCALED in fp8 e4m3
    (values ~N(0,1): ideal e4m3 range; halves the dominant DMA stream
    vs bf16).  The per-edge scale wrc = wts / max(count[col], 1) is
    folded into a host-built fp8 scatter matrix S (one wrc value per
    edge at its dest-node column), so no on-device S-build is needed.
  * mw1_aug is scaled by 2^3 into fp8 (weights ~N(0,0.05) would hit
    e4m3 subnormals); the 2^-3 is folded into Wc = mw2 @ uw1r on host.
  * Device, per 128-edge tile: h = eax.T @ mw1 (fp8 matmul, PSUM),
    relu in 4-tile batches strictly alternating ACT/DVE (the 16.9M-
    element relu saturates either engine alone; fine batches + 4 PSUM
    bufs decouple relu latency from the PE loop, and alternating whole
    batches keeps the two engines on different PSUM bank pairs —
    same-bank concurrent reads serialize), output fp8 g8, scatter
    T_w[hid, node] += g8.T @ S per 128-node window (fp8).  Tile 0 of
    each window scatters the full 128 cols to init PSUM (measured
    faster than trimming to the live span).
  * Update MLP per 4-window quad: h2 = uw1aug.T @ xcon + Wc.T @ T,
    relu (DVE), out = uw2.T @ h2r, ub2 added via ACT bias, written
    bf16.  hh copy on ACT.
  * All heavy input DMA goes through the gpsimd software DGE so
    packets spread across all 16 DMA engines; output DMAs issue from
    the (otherwise idle) sync engine's queue so the gpsimd drain at
    teardown overlaps the body.  A small first eax group + a 4-way
    split of the xcon stream shorten the startup bubble; S is queued
    before eax within each group (scatters need it ~1 batch after
    the messages).
"""
import numpy as np
import ml_dtypes

import concourse.bacc as bacc
import concourse.tile as tile
from concourse import mybir
from concourse.bass_utils import run_bass_kernel_spmd

BF = mybir.dt.bfloat16
F32 = mybir.dt.float32
F8 = mybir.dt.float8e4
bf16 = ml_dtypes.bfloat16
f8 = ml_dtypes.float8_e4m3

P = 128
NCORES = 8
HID = 128
NODE_D = 64
EDGE_D = 32
GLOB_D = 32
FEAT = 98                    # x | ea | 1 | zero-pad
XCON_R = 98                  # x | u | s | ones
K_MW1 = 8.0                  # fp8 scale for mw1 (folded out via Wc)
GROUP = 64                   # 128-edge tiles per eax/S DMA group
GROUP0 = 8                   # tiles in the first (startup) group
RGRP = 4                     # tiles per relu batch (PSUM group)
QUAD = 4                     # windows per update-MLP batch
OGRP = 2                     # quads per output DMA
XCON_SPLIT = 4               # xcon DMA pieces (issued after groups 1..n)
N_WARM = 0                   # startup PE warm-up matmuls (HAM pre-ramp)

# const blob column layout (bf16)
_B_UW1 = 0                   # [0:98, 0:128]      uw1aug (x|u|v_row|ub1)
_B_WC = 128                  # [0:128, 128:256]   Wc = mw2 @ uw1r / K_MW1
_B_UW2 = 256                 # [0:128, 256:320]   uw2
BLOB_W = 320

CFG = {
    "geax": 4, "gsc": 4, "gg": 8, "gn": 4, "go": 3,
    "ph": 4, "pt": 2, "p2": 1,
    "h2r_act": False,          # h2r relu on ACT (else DVE)
    "hh_act": True,          # hh copy on ACT (else DVE)
    "out_act": True,          # out bias-add on ACT (else DVE)
}

_program_cache: dict = {}
_last_results = None


def _build_program(t_sched, ranges):
    nt = sum(t_sched)
    e_pad = nt * P
    nslots = len(t_sched)
    nsh = nslots * P

    # flat per-tile [lo, hi) and S column offsets (compile-time)
    rflat = []
    for j in range(nslots):
        rflat.extend(ranges[j])
    soff = [0]
    for lo, hi in rflat:
        soff.append(soff[-1] + (hi - lo))
    s_tot = soff[-1]

    nc = bacc.Bacc()
    eax_d = nc.dram_tensor("eax", [FEAT, e_pad], F8, kind="ExternalInput")
    sw_d = nc.dram_tensor("sw", [P, s_tot], F8, kind="ExternalInput")
    mw1_d = nc.dram_tensor("mw1", [FEAT, HID], F8, kind="ExternalInput")
    blob_d = nc.dram_tensor("blob", [P, BLOB_W], BF, kind="ExternalInput")
    ub2_d = nc.dram_tensor("ub2", [64, 1], F32, kind="ExternalInput")
    xcon_d = nc.dram_tensor("xcon", [XCON_R, nsh], BF, kind="ExternalInput")
    out_d = nc.dram_tensor("out", [64, nsh], BF, kind="ExternalOutput")

    # flat tile metadata: (slot j, ti within slot, tj, lo, hi)
    tmeta = []
    for j in range(nslots):
        for ti in range(t_sched[j]):
            lo, hi = ranges[j][ti]
            tmeta.append((j, ti, t_sched[j], lo, hi))

    # group starts: small first group for fast pipeline startup
    gstarts = [0]
    while gstarts[-1] < nt:
        gstarts.append(min(nt, gstarts[-1] + (GROUP0 if len(gstarts) == 1
                                              else GROUP)))
    ngrp = len(gstarts) - 1
    tile_grp = np.zeros(nt, np.int64)
    for gi in range(ngrp):
        tile_grp[gstarts[gi]:gstarts[gi + 1]] = gi

    # max S-group width (cols) for the gsc pool tile size
    sgw_max = 0
    for gi in range(ngrp):
        sgw_max = max(sgw_max, soff[gstarts[gi + 1]] - soff[gstarts[gi]])
    sgw_max = -(-sgw_max // 64) * 64

    with tile.TileContext(nc) as tc:
        with (
            tc.tile_pool(name="consts", bufs=1) as consts,
            tc.tile_pool(name="geax", bufs=CFG["geax"]) as geax,
            tc.tile_pool(name="gsc", bufs=CFG["gsc"]) as gsc,
            tc.tile_pool(name="gg", bufs=CFG["gg"]) as gg,
            tc.tile_pool(name="gn", bufs=CFG["gn"]) as gn,
            tc.tile_pool(name="go", bufs=CFG["go"]) as go,
            tc.tile_pool(name="ph", bufs=CFG["ph"], space="PSUM") as ph,
            tc.tile_pool(name="pt", bufs=CFG["pt"], space="PSUM") as pt,
            tc.tile_pool(name="p2", bufs=CFG["p2"], space="PSUM") as p2,
            tc.tile_pool(name="po", bufs=1, space="PSUM") as po,
        ):
            mw1_t = consts.tile([FEAT, HID], F8)
            nc.gpsimd.dma_start(mw1_t[:], mw1_d[:])
            blob_t = consts.tile([P, BLOB_W], BF)
            nc.gpsimd.dma_start(blob_t[:], blob_d[:])
            uw1_t = blob_t[0:XCON_R, _B_UW1:_B_UW1 + HID]
            wc_t = blob_t[0:HID, _B_WC:_B_WC + HID]
            uw2_t = blob_t[0:HID, _B_UW2:_B_UW2 + 64]
            ub2_t = consts.tile([64, 1], F32)
            nc.gpsimd.dma_start(ub2_t[:], ub2_d[:])
            xcon_t = consts.tile([XCON_R, nsh], BF)
            xc_step = -(-nsh // XCON_SPLIT // 64) * 64
            xc_done = 0

            warm_t = None
            if N_WARM:
                warm_t = consts.tile([P, 512], BF, tag="warm")
                nc.gpsimd.memset(warm_t[:], 0.0)

            def update_mlp(q, qw, last, ptq):
                w = qw * P
                n0 = q * QUAD * P
                hh4 = gn.tile([HID, QUAD * P], BF, tag="hh")
                if CFG["hh_act"]:
                    nc.scalar.copy(hh4[:, 0:w], ptq[:, 0:w])
                else:
                    nc.vector.tensor_copy(hh4[:, 0:w], ptq[:, 0:w])
                h2_ps = p2.tile([HID, QUAD * P], F32, space="PSUM")
                nc.tensor.matmul(
                    h2_ps[:, 0:w], lhsT=uw1_t,
                    rhs=xcon_t[:, n0:n0 + w],
                    start=True, stop=False,
                )
                nc.tensor.matmul(
                    h2_ps[:, 0:w], lhsT=wc_t, rhs=hh4[:, 0:w],
                    start=False, stop=True,
                )
                h2r = gn.tile([HID, QUAD * P], BF, tag="h2r")
                if CFG["h2r_act"]:
                    nc.scalar.activation(
                        h2r[:, 0:w], h2_ps[:, 0:w],
                        mybir.ActivationFunctionType.Relu,
                    )
                else:
                    nc.vector.tensor_scalar_max(
                        h2r[:, 0:w], h2_ps[:, 0:w], 0.0)
                o_ps = po.tile([64, QUAD * P], F32, space="PSUM")
                nc.tensor.matmul(o_ps[:, 0:w], lhsT=uw2_t,
                                 rhs=h2r[:, 0:w], start=True, stop=True)
                qo = q % OGRP
                if qo == 0:
                    self_o = go.tile([64, OGRP * QUAD * P], BF, tag="o")
                    update_mlp.o_sb = self_o
                o_sb = update_mlp.o_sb
                if CFG["out_act"]:
                    nc.scalar.activation(
                        o_sb[:, qo * QUAD * P:qo * QUAD * P + w],
                        o_ps[:, 0:w],
                        mybir.ActivationFunctionType.Identity,
                        bias=ub2_t[:, 0:1],
                    )
                else:
                    nc.vector.tensor_scalar(
                        out=o_sb[:, qo * QUAD * P:qo * QUAD * P + w],
                        in0=o_ps[:, 0:w],
                        scalar1=ub2_t[:, 0:1], scalar2=None,
                        op0=mybir.AluOpType.add,
                    )
                if qo == OGRP - 1 or last:
                    ow = (q - qo) * QUAD * P
                    nc.sync.dma_start(
                        out_d[:, ow:n0 + w],
                        o_sb[:, 0:n0 + w - ow],
                    )

            eax_g = None
            sw_g = None
            sg0 = 0
            gprev = -1
            ptq = None
            t = 0
            while t < nt:
                nr = min(RGRP, nt - t)
                t0 = t
                h8_ps = ph.tile([P, RGRP * HID], F32, space="PSUM")
                if t0 == 0 and N_WARM:
                    # PE warm-up: dummy matmuls on (uninitialized) SBUF
                    # into the first h8 tile, overwritten by the real
                    # start=True message matmuls below.  Keeps PE busy
                    # through the startup DMA window so HAM reaches
                    # full clock (K=8/8) before real work arrives.
                    for _ in range(N_WARM):
                        nc.tensor.matmul(
                            h8_ps[:, 0:RGRP * HID],
                            lhsT=warm_t[:, 0:P],
                            rhs=warm_t[:, 0:RGRP * HID],
                            start=True, stop=True, skip_group_check=True,
                        )
                srefs = []
                for i in range(nr):
                    gi = int(tile_grp[t])
                    if gi != gprev:
                        gprev = gi
                        ga, gb = gstarts[gi], gstarts[gi + 1]
                        sg0 = soff[ga]
                        sg1 = soff[gb]
                        sw_g = gsc.tile([P, sgw_max], F8, tag="sw")
                        nc.sync.dma_start(
                            sw_g[:, 0:sg1 - sg0], sw_d[:, sg0:sg1])
                        eax_g = geax.tile([FEAT, GROUP * P], F8, tag="eax")
                        nc.gpsimd.dma_start(
                            eax_g[:, 0:(gb - ga) * P],
                            eax_d[:, ga * P:gb * P],
                        )
                        if gi >= 1 and xc_done < nsh:
                            xe = min(xc_done + xc_step, nsh)
                            nc.gpsimd.dma_start(
                                xcon_t[:, xc_done:xe],
                                xcon_d[:, xc_done:xe])
                            xc_done = xe
                    r = t - gstarts[gi]
                    nc.tensor.matmul(
                        h8_ps[:, i * HID:(i + 1) * HID],
                        lhsT=eax_g[:, r * P:(r + 1) * P],
                        rhs=mw1_t,
                        start=True, stop=True,
                    )
                    srefs.append((sw_g, soff[t] - sg0))
                    t += 1
                # relu whole batch on one engine, alternating ACT/DVE
                # per batch: concurrent batches then use different PSUM
                # bank pairs (same-bank reads by both engines serialize)
                g8a = gg.tile([P, RGRP * HID], F8, tag="Ga")
                if (t0 // RGRP) % 2 == 1:
                    nc.vector.tensor_scalar_max(
                        g8a[:, 0:nr * HID], h8_ps[:, 0:nr * HID], 0.0)
                else:
                    nc.scalar.activation(
                        g8a[:, 0:nr * HID], h8_ps[:, 0:nr * HID],
                        mybir.ActivationFunctionType.Relu,
                    )
                for i in range(nr):
                    j, ti, tj, lo, hi = tmeta[t0 + i]
                    q, jr = divmod(j, QUAD)
                    if jr == 0 and ti == 0:
                        ptq = pt.tile([P, QUAD * P], F32, space="PSUM")
                    sw_i, so = srefs[i]
                    nc.tensor.matmul(
                        ptq[:, jr * P + lo:jr * P + hi],
                        lhsT=g8a[:, i * HID:(i + 1) * HID],
                        rhs=sw_i[:, so:so + (hi - lo)],
                        start=(ti == 0), stop=(ti == tj - 1),
                        skip_group_check=True,
                    )
                    if ti == tj - 1 and (jr == QUAD - 1 or j == nslots - 1):
                        qw = min(QUAD, nslots - q * QUAD)
                        update_mlp(q, qw, j == nslots - 1, ptq)
    nc.finalize()
    return nc


def _schedule(col, n_nodes):
    """Assign 128-node blocks to (core, slot) and derive the shared
    per-slot tile schedule."""
    nblk = -(-n_nodes // P)
    nslots = -(-nblk // NCORES)
    nblk_pad = nslots * NCORES
    nsh = nslots * P

    blk = (col >> 7).astype(np.int64)
    order = np.argsort(col, kind="stable")
    bc = np.bincount(blk, minlength=nblk_pad)
    bstart = np.zeros(nblk_pad + 1, np.int64)
    np.cumsum(bc, out=bstart[1:])

    sorted_blocks = np.argsort(-bc, kind="stable")
    blk_assign = sorted_blocks.reshape(nslots, NCORES)   # [slot, core]
    grp_max = bc[blk_assign].max(axis=1)
    t_sched = [int(v) for v in np.maximum(1, -(-grp_max // P))]
    return t_sched, blk_assign, order, bc, bstart, nslots, nsh


def kernel(x, edge_index, edge_attr, u, node_batch, wts,
           mw1, mb1, mw2, mb2, uw1, ub1, uw2, ub2):
    x = np.asarray(x, np.float32)
    edge_index = np.asarray(edge_index)
    edge_attr = np.asarray(edge_attr, np.float32)
    u = np.asarray(u, np.float32)
    node_batch = np.asarray(node_batch).astype(np.int64)
    wts = np.asarray(wts, np.float32).reshape(-1)
    mw1 = np.asarray(mw1, np.float32)
    mb1 = np.asarray(mb1, np.float32)
    mw2 = np.asarray(mw2, np.float32)
    mb2 = np.asarray(mb2, np.float32)
    uw1 = np.asarray(uw1, np.float32)
    ub1 = np.asarray(ub1, np.float32)
    uw2 = np.asarray(uw2, np.float32)
    ub2 = np.asarray(ub2, np.float32)

    n_nodes = x.shape[0]
    row = np.asarray(edge_index[0], np.int64)
    col = np.asarray(edge_index[1], np.int64)

    sched = _schedule(col, n_nodes)
    (t_sched, blk_assign, order, bc, bstart, nslots, nsh) = sched
    nt = sum(t_sched)
    e_pad = nt * P

    # per-node stats (host): count, 1/max(cnt,1), weight-sum
    cnt = np.bincount(col, minlength=n_nodes).astype(np.float32)
    rc = 1.0 / np.maximum(cnt, 1.0)
    wsum = np.bincount(col, weights=wts, minlength=n_nodes).astype(np.float32)
    s_node = wsum * rc

    # per-edge
    colof = (col & 127).astype(np.int64)
    wrc = wts * rc[col]

    slot_off0 = np.zeros(nslots + 1, np.int64)
    np.cumsum(np.asarray(t_sched) * P, out=slot_off0[1:])

    # per-core edge slots; ranges = per-(slot,tile) union col span
    core_idx = []
    rlo = np.full(nt, P, np.int64)
    rhi = np.zeros(nt, np.int64)
    for c in range(NCORES):
        eidx = np.full(e_pad, -1, np.int64)
        nidx = np.full(nsh, -1, np.int64)
        for j in range(nslots):
            b = int(blk_assign[j, c])
            m = int(bc[b])
            o = slot_off0[j]
            eidx[o:o + m] = order[bstart[b]:bstart[b] + m]
            n0 = b * P
            nn = min(P, n_nodes - n0)
            if nn > 0:
                nidx[j * P:j * P + nn] = np.arange(n0, n0 + nn)
        evalid = eidx >= 0
        eidxc = np.where(evalid, eidx, 0)
        cof = np.where(evalid, colof[eidxc], -1).astype(np.int64)
        cof_t = cof.reshape(nt, P)
        vm = cof_t >= 0
        np.minimum(rlo, np.where(vm, cof_t, P).min(axis=1), out=rlo)
        np.maximum(rhi, np.where(vm, cof_t, -1).max(axis=1) + 1, out=rhi)
        core_idx.append((eidx, evalid, eidxc, nidx, cof))
    rlo = np.minimum(rlo, rhi)           # empty tiles -> [hi, hi)
    # Seamless coverage: every PSUM column in [0,128) must be written by
    # some tile of its window (per-element has_written), so stretch tile 0
    # down to 0, the last tile up to 128, and close inter-tile gaps.
    ranges = []
    for j in range(nslots):
        t0 = slot_off0[j] // P
        tj = t_sched[j]
        lo = [int(rlo[t0 + k]) for k in range(tj)]
        hi = [int(rhi[t0 + k]) for k in range(tj)]
        lo[0], hi[0] = 0, P
        for k in range(1, tj):
            hi[k] = max(hi[k], lo[k])
        ranges.append(list(zip(lo, hi)))

    key = (tuple(t_sched),
           tuple(v for rj in ranges for lh in rj for v in lh))
    if key not in _program_cache:
        _program_cache[key] = _build_program(t_sched, ranges)
    nc = _program_cache[key]

    # flat ranges + S col offsets (must match _build_program)
    rflat = [lh for rj in ranges for lh in rj]
    wd_flat = np.array([hi - lo for lo, hi in rflat], np.int64)
    lo_flat = np.array([lo for lo, hi in rflat], np.int64)
    soff = np.zeros(nt + 1, np.int64)
    np.cumsum(wd_flat, out=soff[1:])
    s_tot = int(soff[-1])

    # consts (shared by all cores)
    v_row = mb2 @ uw1[NODE_D:2 * NODE_D, :]              # [HID]
    wc = (mw2 @ uw1[NODE_D:2 * NODE_D, :]) / K_MW1       # [HID, HID]
    blob = np.zeros((P, BLOB_W), np.float32)
    blob[0:NODE_D, _B_UW1:_B_UW1 + HID] = uw1[0:NODE_D, :]
    blob[NODE_D:NODE_D + GLOB_D, _B_UW1:_B_UW1 + HID] = uw1[2 * NODE_D:, :]
    blob[NODE_D + GLOB_D, _B_UW1:_B_UW1 + HID] = v_row
    blob[NODE_D + GLOB_D + 1, _B_UW1:_B_UW1 + HID] = ub1
    blob[0:HID, _B_WC:_B_WC + HID] = wc
    blob[0:HID, _B_UW2:_B_UW2 + 64] = uw2
    blob_bf = blob.astype(bf16)
    ub2_a = ub2.reshape(64, 1).astype(np.float32)

    mw1_aug = np.zeros((FEAT, HID), np.float32)
    mw1_aug[0:NODE_D + EDGE_D] = mw1 * K_MW1
    mw1_aug[NODE_D + EDGE_D] = mb1 * K_MW1
    mw1_f8 = mw1_aug.astype(f8)

    u_per_node = u[node_batch]                           # [N, GLOB_D]

    in_maps = []
    node_idx_cores = []
    for c in range(NCORES):
        eidx, evalid, eidxc, nidx, cof = core_idx[c]
        # eax: [x[row] | edge_attr | 1] unscaled fp8, zeros on pads
        eax = np.empty((e_pad, FEAT), np.float32)
        eax[:, 0:NODE_D] = x[row[eidxc]]
        eax[:, NODE_D:NODE_D + EDGE_D] = edge_attr[eidxc]
        eax[:, NODE_D + EDGE_D] = 1.0
        eax[:, FEAT - 1] = 0.0
        eax[~evalid] = 0.0

        # S: [128, s_tot] fp8; S[p, soff[t] + colof - lo[t]] = wrc
        tvec = np.arange(e_pad) >> 7
        pvec = np.arange(e_pad) & 127
        scol = soff[tvec] + cof - lo_flat[tvec]
        sarr = np.zeros((P, s_tot), np.float32)
        sarr[pvec[evalid], scol[evalid]] = wrc[eidxc[evalid]]

        nvalid = nidx >= 0
        nidxc = np.where(nvalid, nidx, 0)
        xcon = np.zeros((nsh, XCON_R), np.float32)
        xcon[:, 0:NODE_D] = x[nidxc]
        xcon[:, NODE_D:NODE_D + GLOB_D] = u_per_node[nidxc]
        xcon[:, NODE_D + GLOB_D] = s_node[nidxc]
        xcon[:, NODE_D + GLOB_D + 1] = 1.0
        xcon[~nvalid] = 0.0

        in_maps.append({
            "eax": np.ascontiguousarray(eax.T).astype(f8),
            "sw": sarr.astype(f8),
            "mw1": mw1_f8,
            "blob": blob_bf,
            "ub2": ub2_a,
            "xcon": np.ascontiguousarray(xcon.T).astype(bf16),
        })
        node_idx_cores.append((nidx, nvalid))

    res = run_bass_kernel_spmd(nc, in_maps, core_ids=list(range(NCORES)))
    global _last_results
    _last_results = res

    out_full = np.zeros((n_nodes, 64), np.float32)
    for c in range(NCORES):
        nidx, nvalid = node_idx_cores[c]
        oc = np.asarray(res.results[c]["out"], np.float32)   # [64, nsh]
        out_full[nidx[nvalid]] = oc.T[nvalid]
    return out_full
